# revision 1
# baseline (speedup 1.0000x reference)
"""Trainium2 Bass kernel for nn_CrossAttention_9174050144362.

Reference computation (per batch b, spatial flattened to hw=4096):
    Q = Wq @ a + bq      [128, 4096]
    K = Wk @ p + bk      [128, 4096]
    V = Wv @ p + bv      [256, 4096]
    attn = softmax_n(Q^T K)            [4096, 4096]
    out  = V @ attn^T + a              [256, 4096]

Sharding: 8 cores = (4 batches) x (2 query halves of 2048). Each core
computes full K/V for its batch and attends its 2048 queries against all
4096 keys. No collectives.

v3 schedule:
  * Flash-style prefix: chunk-0 S/exp/VP runs per p-eighth as the DMA
    lands; a-quarters are front-loaded on the scalar ring so the Q
    projections never stall the PE; V^T tiles are evicted in pairs on
    the DVE so the ACT engine only carries the exps.
  * S tiles are computed in pairs into a 2-bank PSUM slot and exp'd by
    ONE activation over [P,1024] - the ACT engine drops from 22us to
    16.6us per chunk, giving the PE slack.
  * exp outputs land in a persistent per-chunk pt store [32 tiles]; the
    softmax denominator is a bf16 in-place pair/quad/oct cascade on the
    DVE (2x mode), finishing ~0.9us after the last exp.
  * partition-reduce + broadcast of the denominator in one ones-matmul
    (PSUM slot borrowed from the S pool); 1/x via
    reciprocal_approx_fast; epilogue of chunk c threads through chunk
    c+1. PSUM: 2x2 S banks + 2x2 out banks = 8.
"""

import numpy as np

import concourse.bass as bass
import concourse.tile as tile
from concourse import bacc, mybir
from concourse.bass_utils import run_bass_kernel_spmd

B, C, H, W = 4, 256, 64, 64
HW = H * W            # 4096 keys
CH = C // 2           # 128 q/k channels
P = 128               # partitions
MS = HW // 2          # 2048 queries per core
MCH = 512             # query chunk (PSUM-bank sized)
NT = HW // P          # 32 key tiles
NCH = MS // MCH       # 4 query chunks
NCORES = 8

F32 = mybir.dt.float32
F32R = mybir.dt.float32r
BF16 = mybir.dt.bfloat16
AF = mybir.ActivationFunctionType

TRACE = False
TMPDIR = None
LAST_RESULT = None

_PROG = None
_ONES = np.ones((P, P), dtype=np.float32)


def _emit(tc, out_d, a_d, p_d, wqt_d, wkt_d, wvt_d, bq_d, bk_d, bv_d, ones_d):
    nc = tc.nc
    ts = bass.ts

    with (
        tc.tile_pool(name="statics", bufs=1) as statics,
        tc.tile_pool(name="rcp", bufs=2) as rcp,
        tc.tile_pool(name="osb", bufs=4) as osb,
        tc.tile_pool(name="psS", bufs=3, space="PSUM") as psS,
        tc.tile_pool(name="psOut", bufs=1, space="PSUM") as psOut,
    ):
        # ---- small statics (sync queue, land first)
        wqt_sb = statics.tile([P, 2, CH], F32R)
        nc.sync.dma_start(wqt_sb[:], wqt_d.rearrange("(co ci) o -> ci co o", ci=P))
        wkt_sb = statics.tile([P, 2, CH], F32R)
        nc.sync.dma_start(wkt_sb[:], wkt_d.rearrange("(co ci) o -> ci co o", ci=P))
        wvt_sb = statics.tile([P, 2, C], F32R)
        nc.sync.dma_start(wvt_sb[:], wvt_d.rearrange("(co ci) o -> ci co o", ci=P))
        bq_sb = statics.tile([P, 1], F32)
        nc.sync.dma_start(bq_sb[:], bq_d[:])
        bk_sb = statics.tile([P, 1], F32)
        nc.sync.dma_start(bk_sb[:], bk_d[:])
        bv_sb = statics.tile([P, 2], F32)
        nc.sync.dma_start(bv_sb[:], bv_d[:])
        ones_sb = statics.tile([P, P], F32R)
        nc.sync.dma_start(ones_sb[:], ones_d[:])

        # ---- bulk inputs. gpsimd ring: even p-eighths. scalar ring: a0
        # first (unblocks Q proj), then early p odds, then the rest of a
        # (all a is resident by ~18us; chunk>=1 projections sit late in
        # the prefix), then the late p odds.
        a_v = a_d.rearrange("(co ci) m -> ci co m", ci=P)
        a_sb = statics.tile([P, 2, MS], F32R)
        p_v = p_d.rearrange("(co ci) m -> ci co m", ci=P)
        p_sb = statics.tile([P, 2, HW], F32R)
        E8 = HW // 8
        # 1MB pieces amortize the ~2us fixed DMA cost; 3 rings in
        # parallel land everything by ~16us. p quarters alternate
        # gpsimd/vector rings so consecutive eighths arrive in order.
        # each p-quarter is split across the gpsimd and sync rings by
        # partition range, so quarters complete in consumption order at
        # the two rings' aggregate bandwidth.
        Q4 = HW // 4
        for i in range(4):
            nc.gpsimd.dma_start(p_sb[0:64, :, ts(i, Q4)], p_v[0:64, :, ts(i, Q4)])
            nc.sync.dma_start(p_sb[64:128, :, ts(i, Q4)], p_v[64:128, :, ts(i, Q4)])
        nc.scalar.dma_start(a_sb[:, :, ts(0, MS // 2)], a_v[:, :, ts(0, MS // 2)])
        nc.scalar.dma_start(a_sb[:, :, ts(1, MS // 2)], a_v[:, :, ts(1, MS // 2)])

        # ---- persistent SBUF state
        q_sb = statics.tile([P, MS], F32R)
        k_sb = statics.tile([P, HW], F32R)
        vt_sb = statics.tile([P, NT, C], BF16)
        # per-chunk exp store, 2 rotating slots; the den cascade reduces
        # in place over tile slots (pair->quad->oct->final).
        pt_sb = statics.tile([P, 2, NT, MCH], BF16)
        acc_sb = statics.tile([P, 2, MCH], F32R)   # final den accumulators
        hlf_sb = statics.tile([P, 2, 2, MCH], BF16)  # oct-merge scratch
        ab2_sb = statics.tile([P, 2, MS], F32)     # a + bv (epilogue residual)

        out_v = out_d.rearrange("(co ci) m -> ci co m", ci=P)

        # ---------- helpers ----------
        # Projections pack into HALVES of the [P,2,MCH] psS slots so the
        # prefix only makes 4 slot allocations per p-eighth (2 bufs
        # rotate cleanly). K/Q biases are applied by the evicting engine.
        def proj_q(c, half):
            for co in range(2):
                nc.tensor.matmul(half[:], wqt_sb[:, co, :],
                                 a_sb[:, co, ts(c, MCH)],
                                 start=(co == 0), stop=(co == 1))
            nc.scalar.activation(q_sb[:, ts(c, MCH)], half[:], AF.Identity,
                                 bias=bq_sb[:, 0:1])

        def proj_k(e, half):
            for co in range(2):
                nc.tensor.matmul(half[:], wkt_sb[:, co, :],
                                 p_sb[:, co, ts(e, E8)],
                                 start=(co == 0), stop=(co == 1))
            nc.vector.tensor_scalar_add(k_sb[:, ts(e, E8)], half[:],
                                        bk_sb[:, 0:1])

        def proj_vt_pair(u, half):
            # V^T tiles 2u, 2u+1 into one [P,MCH] psum half; one DVE
            # eviction. vt[n, c] = sum_ci p[ci, n] * WvT[ci, c]  (no bias;
            # bv folded into ab2 since attn rows sum to 1)
            for i in range(2):
                t = 2 * u + i
                for co in range(2):
                    nc.tensor.matmul(half[:, ts(i, C)], p_sb[:, co, ts(t, P)],
                                     wvt_sb[:, co, :],
                                     start=(co == 0), stop=(co == 1))
            nc.vector.tensor_copy(
                vt_sb[:, 2 * u : 2 * u + 2, :],
                half[:].rearrange("p (t c) -> p t c", t=2))

        def s_pair(c, j):
            # S tiles 2j, 2j+1 -> one [P,2,512] PSUM slot, ONE exp over
            # [P,1024] into the pt store.
            s = c % 2
            ps = psS.tile([P, 2, MCH], F32, tag="ps", name="ps_s")
            for i in range(2):
                t = 2 * j + i
                nc.tensor.matmul(ps[:, i, :], k_sb[:, ts(t, P)],
                                 q_sb[:, ts(c, MCH)], start=True, stop=True)
            nc.scalar.activation(pt_sb[:, s, 2 * j : 2 * j + 2, :], ps[:], AF.Exp)

        def vp(c, t, out_ps, last=False):
            s = c % 2
            for co in range(2):
                nc.tensor.matmul(out_ps[:, co, :],
                                 vt_sb[:, t, ts(co, P)], pt_sb[:, s, t, :],
                                 start=(t == 0), stop=last)

        # --- denominator cascade (all DVE, bf16 2x, in place) ---
        def pair(c, j):
            s = c % 2
            nc.vector.tensor_add(pt_sb[:, s, 2 * j, :], pt_sb[:, s, 2 * j, :],
                                 pt_sb[:, s, 2 * j + 1, :])

        def quad(c, k):
            s = c % 2
            nc.vector.tensor_add(pt_sb[:, s, 4 * k, :], pt_sb[:, s, 4 * k, :],
                                 pt_sb[:, s, 4 * k + 2, :])

        def oct(c, o):
            s = c % 2
            nc.vector.tensor_add(pt_sb[:, s, 8 * o, :], pt_sb[:, s, 8 * o, :],
                                 pt_sb[:, s, 8 * o + 4, :])

        def den_steps(c, j):
            # after pair j completes, fold finished subtree sums into a
            # running total hlf so the post-vp(31) chain is only
            # pair15 -> quad7 -> fin (~1.6us), not a 5-op tree.
            s = c % 2
            pair(c, j)
            if j % 2 == 1:
                quad(c, j // 2)
            if j % 4 == 3 and j < 14:
                oct(c, j // 4)
            if j == 7:    # hlf = tiles 0..15
                nc.vector.tensor_add(hlf_sb[:, s, 0, :], pt_sb[:, s, 0, :],
                                     pt_sb[:, s, 8, :])
            elif j == 12:  # hlf += tiles 16..23
                nc.vector.tensor_add(hlf_sb[:, s, 0, :], hlf_sb[:, s, 0, :],
                                     pt_sb[:, s, 16, :])
            elif j == 14:  # hlf += tiles 24..27
                nc.vector.tensor_add(hlf_sb[:, s, 0, :], hlf_sb[:, s, 0, :],
                                     pt_sb[:, s, 24, :])
            elif j == 15:  # acc = hlf + tiles 28..31
                nc.vector.tensor_add(acc_sb[:, s, :], hlf_sb[:, s, 0, :],
                                     pt_sb[:, s, 28, :])

        def den_mm(c):
            # ones^T @ acc: reduces over partitions AND broadcasts row-sums
            den_ps = psS.tile([P, 2, MCH], F32, tag="ps", name="ps_den")
            nc.tensor.matmul(den_ps[:, 0, :], ones_sb[:], acc_sb[:, c % 2, :],
                             start=True, stop=True)
            return den_ps

        def recip_of(den_ps):
            r = rcp.tile([P, MCH], F32, tag="rc")
            nc.vector.reciprocal_approx_fast(out=r[:], in_=den_ps[:, 0, :])
            return r

        def ep_mul(pout, r, co):
            o = osb.tile([P, MCH], F32, tag="osb")
            nc.vector.tensor_mul(o[:], pout[:, co, :], r[:])
            return o

        def ep_add_dma(c, o, co, q=nc.sync):
            nc.vector.tensor_add(o[:], o[:], ab2_sb[:, co, ts(c, MCH)])
            q.dma_start(out_v[:, co, ts(c, MCH)], o[:])

        def ab2():
            for co in range(2):
                nc.vector.tensor_scalar_add(ab2_sb[:, co, :], a_sb[:, co, :],
                                            bv_sb[:, co:co + 1])

        # ---------- prefix: projections + S/exp of chunk 0 ----------
        # Only ~19us of PE work: it hides fully under the ~20us input
        # stream. VP is deferred (pt persists), so psOut stays 1 slot and
        # psS gets 3 pair-slots of elasticity.
        for e in range(8):
            psA = psS.tile([P, 2, MCH], F32, tag="ps", name="ps_proj_a")
            if e == 0:
                proj_q(0, psA[:, 1, :])
            elif e == 4:
                proj_q(1, psA[:, 1, :])
            proj_k(e, psA[:, 0, :])
            psB = psS.tile([P, 2, MCH], F32, tag="ps", name="ps_proj_b")
            proj_vt_pair(2 * e, psB[:, 0, :])
            proj_vt_pair(2 * e + 1, psB[:, 1, :])
            s_pair(0, 2 * e)
            s_pair(0, 2 * e + 1)
        ab2()

        # ---------- phases X_c: VP of chunk c + S/exp of chunk c+1 ----
        # (+ epilogue of chunk c-1 at phase start)
        st = {}
        for c in range(NCH):
            # Phase start: chunk c's last 3 S pairs (deferred from the
            # previous phase) give the PE ~6us of epilogue-independent
            # work while chunk c-1's epilogue (den matmul, 1/x,
            # normalize, +a+bv) drains and frees the psOut slot.
            if 1 <= c < NCH - 1:
                psQ = psS.tile([P, 2, MCH], F32, tag="ps", name="ps_q")
                proj_q(c + 1, psQ[:, 0, :])
            if c >= 1:
                s_pair(c, 13)
                s_pair(c, 14)
                st["den"] = den_mm(c - 1)
                s_pair(c, 15)
                st["rc"] = recip_of(st.pop("den"))
                pout = st.pop("pout")
                st["o0"] = ep_mul(pout, st["rc"], 0)
                st["o1"] = ep_mul(pout, st.pop("rc"), 1)
                ep_add_dma(c - 1, st.pop("o0"), 0)
                ep_add_dma(c - 1, st.pop("o1"), 1)
            out_ps = psOut.tile([P, 2, MCH], F32, tag="out")
            for j in range(NT // 2):
                vp(c, 2 * j, out_ps)
                vp(c, 2 * j + 1, out_ps, last=(j == NT // 2 - 1))
                den_steps(c, j)
                if c < NCH - 1 and j <= 12:
                    s_pair(c + 1, j)
            st["pout"] = out_ps

        # ---------- tail: chunk 3 epilogue (per-co pipelined) ----------
        pc = NCH - 1
        pout = st.pop("pout")
        den_ps = den_mm(pc)
        r = recip_of(den_ps)
        o0 = ep_mul(pout, r, 0)
        ep_add_dma(pc, o0, 0, q=nc.sync)
        o1 = ep_mul(pout, r, 1)
        ep_add_dma(pc, o1, 1, q=nc.scalar)


def _build():
    nc = bacc.Bacc("TRN2", target_bir_lowering=False, debug=False)
    a_d = nc.dram_tensor("a_s", [C, MS], F32R, kind="ExternalInput").ap()
    p_d = nc.dram_tensor("p_s", [C, HW], F32R, kind="ExternalInput").ap()
    wqt_d = nc.dram_tensor("wqt", [C, CH], F32R, kind="ExternalInput").ap()
    wkt_d = nc.dram_tensor("wkt", [C, CH], F32R, kind="ExternalInput").ap()
    wvt_d = nc.dram_tensor("wvt", [C, C], F32R, kind="ExternalInput").ap()
    bq_d = nc.dram_tensor("bq", [CH, 1], F32, kind="ExternalInput").ap()
    bk_d = nc.dram_tensor("bk", [CH, 1], F32, kind="ExternalInput").ap()
    bv_d = nc.dram_tensor("bv", [P, 2], F32, kind="ExternalInput").ap()
    ones_d = nc.dram_tensor("onesm", [P, P], F32R, kind="ExternalInput").ap()
    out_d = nc.dram_tensor("out_s", [C, MS], F32, kind="ExternalOutput").ap()
    with tile.TileContext(nc) as tc:
        _emit(tc, out_d, a_d, p_d, wqt_d, wkt_d, wvt_d, bq_d, bk_d, bv_d, ones_d)
    nc.compile()
    return nc


def _get_prog():
    global _PROG
    if _PROG is None:
        _PROG = _build()
    return _PROG


def kernel(**inputs):
    a = np.ascontiguousarray(np.asarray(inputs["a"], dtype=np.float32)).reshape(
        B, C, HW
    )
    p = np.ascontiguousarray(np.asarray(inputs["p"], dtype=np.float32)).reshape(
        B, C, HW
    )
    wqt = np.ascontiguousarray(np.asarray(inputs["Wq"], dtype=np.float32).T)
    wkt = np.ascontiguousarray(np.asarray(inputs["Wk"], dtype=np.float32).T)
    wvt = np.ascontiguousarray(np.asarray(inputs["Wv"], dtype=np.float32).T)
    bq = np.ascontiguousarray(np.asarray(inputs["bq"], dtype=np.float32)).reshape(
        CH, 1
    )
    bk = np.ascontiguousarray(np.asarray(inputs["bk"], dtype=np.float32)).reshape(
        CH, 1
    )
    bv = np.ascontiguousarray(
        np.asarray(inputs["bv"], dtype=np.float32).reshape(2, P).T
    )

    nc = _get_prog()
    in_maps = []
    for core in range(NCORES):
        b, h = divmod(core, 2)
        in_maps.append(
            {
                "a_s": np.ascontiguousarray(a[b, :, h * MS : (h + 1) * MS]),
                "p_s": p[b],
                "wqt": wqt,
                "wkt": wkt,
                "wvt": wvt,
                "bq": bq,
                "bk": bk,
                "bv": bv,
                "onesm": _ONES,
            }
        )
    kwargs = {}
    if TRACE:
        kwargs["trace"] = True
        if TMPDIR:
            kwargs["tmpdir"] = TMPDIR
    res = run_bass_kernel_spmd(nc, in_maps, core_ids=list(range(NCORES)), **kwargs)
    global LAST_RESULT
    LAST_RESULT = res

    out = np.empty((B, C, HW), dtype=np.float32)
    for core in range(NCORES):
        b, h = divmod(core, 2)
        out[b, :, h * MS : (h + 1) * MS] = res.results[core]["out_s"]
    return out.reshape(B, C, H, W)



# revision 5
# speedup vs baseline: 1.3320x; 1.3320x over previous
"""Trainium2 Bass kernel for nn_CrossAttention_9174050144362.

Reference computation (per batch b, spatial flattened to hw=4096):
    Q = Wq @ a + bq      [128, 4096]
    K = Wk @ p + bk      [128, 4096]
    V = Wv @ p + bv      [256, 4096]
    attn = softmax_n(Q^T K)            [4096, 4096]
    out  = V @ attn^T + a              [256, 4096]

Sharding: 8 cores = (4 batches) x (2 query halves of 2048). Each core
computes full K/V for its batch and attends its 2048 queries against all
4096 keys. No collectives.

v4 schedule (from v3):
  * All matmul operands bf16 (host-cast): input DMA halves to 3.2MB,
    LDWEIGHTS halve to ~113ns. rel err ~3e-3 (vs 2e-2 budget).
  * Inputs on 4 rings: sync p[q0,q2], gpsimd p[q1,q3], scalar a,
    vector statics - statics no longer delay the sync ring.
  * Q/K PSUM evictions moved to the idle gpsimd engine; ACT does only
    the 64 exps; DVE does vt evictions + den + epilogue.
  * Denominator: 31-op in-place pair/quad/oct cascade replaced by 6
    out-of-place wide adds (3x[P,4096] + [P,2048] + [P,1024] + [P,512],
    bf16 2x) into a scratch - pure reads of pt (no WAR with VP), can
    start as soon as chunk c's exps land (early phase c).
  * den matmul + 1/x run mid-phase; at vp(c,31) only ep_mul remains,
    covered by chunk c+1's 3 deferred S pairs. PSUM: 3x2 S + 1x2 out.
"""

import numpy as np
import ml_dtypes

import concourse.bass as bass
import concourse.tile as tile
from concourse import bacc, mybir
from concourse.bass_utils import run_bass_kernel_spmd

B, C, H, W = 4, 256, 64, 64
HW = H * W            # 4096 keys
CH = C // 2           # 128 q/k channels
P = 128               # partitions
MS = HW // 2          # 2048 queries per core
MCH = 512             # query chunk (PSUM-bank sized)
NT = HW // P          # 32 key tiles
NCH = MS // MCH       # 4 query chunks
NCORES = 8

F32 = mybir.dt.float32
BF16 = mybir.dt.bfloat16
AF = mybir.ActivationFunctionType
BT = ml_dtypes.bfloat16

TRACE = False
TMPDIR = None
LAST_RESULT = None

_PROG = None
_ONES = np.ones((P, P), dtype=BT)


def _emit(tc, out_d, a_d, p_d, wqt_d, wkt_d, wvt_d, bq_d, bk_d, bv_d, ones_d):
    nc = tc.nc
    ts = bass.ts

    with (
        tc.tile_pool(name="statics", bufs=1) as statics,
        tc.tile_pool(name="rcp", bufs=2) as rcp,
        tc.tile_pool(name="osb", bufs=4) as osb,
        tc.tile_pool(name="psS", bufs=3, space="PSUM") as psS,
        tc.tile_pool(name="psOut", bufs=1, space="PSUM") as psOut,
    ):
        # ---- statics first on the scalar ring (sync/gpsimd stay clean
        # for p; a follows the statics on scalar)
        wqt_sb = statics.tile([P, 2, CH], BF16)
        nc.scalar.dma_start(wqt_sb[:], wqt_d[:])
        wkt_sb = statics.tile([P, 2, CH], BF16)
        nc.scalar.dma_start(wkt_sb[:], wkt_d[:])
        wvt_sb = statics.tile([P, 2, C], BF16)
        nc.scalar.dma_start(wvt_sb[:], wvt_d[:])
        bq_sb = statics.tile([P, 1], F32)
        nc.scalar.dma_start(bq_sb[:], bq_d[:])
        bk_sb = statics.tile([P, 1], F32)
        nc.scalar.dma_start(bk_sb[:], bk_d[:])
        bv_sb = statics.tile([P, 2], F32)
        nc.scalar.dma_start(bv_sb[:], bv_d[:])
        ones_sb = statics.tile([P, P], BF16)
        nc.scalar.dma_start(ones_sb[:], ones_d[:])

        # ---- bulk inputs, all bf16. p quarters alternate sync/gpsimd
        # rings; a (2 halves) on the scalar ring.
        a_sb = statics.tile([P, 2, MS], BF16)
        p_sb = statics.tile([P, 2, HW], BF16)
        Q4 = HW // 4
        for i in range(4):
            q = nc.sync if i % 2 == 0 else nc.gpsimd
            q.dma_start(p_sb[:, :, ts(i, Q4)], p_d[:, :, ts(i, Q4)])
        for i in range(2):
            nc.scalar.dma_start(a_sb[:, :, ts(i, MS // 2)],
                                a_d[:, :, ts(i, MS // 2)])

        # ---- persistent SBUF state
        q_sb = statics.tile([P, MS], BF16)
        k_sb = statics.tile([P, HW], BF16)
        vt_sb = statics.tile([P, NT, C], BF16)
        # per-chunk exp store, 2 rotating slots (read-only after write;
        # the den tree reduces into dscr, not in place).
        pt_sb = statics.tile([P, 2, NT, MCH], BF16)
        dscr = statics.tile([P, 8, MCH], BF16)     # den tree scratch
        ab2_sb = statics.tile([P, 2, MS], BF16)    # a + bv (residual)

        out_v = out_d.rearrange("(co ci) m -> ci co m", ci=P)

        # ---------- helpers ----------
        def proj_q(c, half):
            for co in range(2):
                nc.tensor.matmul(half[:], wqt_sb[:, co, :],
                                 a_sb[:, co, ts(c, MCH)],
                                 start=(co == 0), stop=(co == 1))
            nc.scalar.activation(q_sb[:, ts(c, MCH)], half[:], AF.Identity,
                                 bias=bq_sb[:, 0:1])

        def proj_k(e, half):
            E8 = HW // 8
            for co in range(2):
                nc.tensor.matmul(half[:], wkt_sb[:, co, :],
                                 p_sb[:, co, ts(e, E8)],
                                 start=(co == 0), stop=(co == 1))
            nc.vector.tensor_scalar_add(k_sb[:, ts(e, E8)], half[:],
                                        bk_sb[:, 0:1])

        def proj_vt_pair(u, half):
            # V^T tiles 2u, 2u+1 into one [P,MCH] psum half; one DVE
            # eviction. vt[n, c] = sum_ci p[ci, n] * WvT[ci, c]  (no bias;
            # bv folded into ab2 since attn rows sum to 1)
            for i in range(2):
                t = 2 * u + i
                for co in range(2):
                    nc.tensor.matmul(half[:, ts(i, C)], p_sb[:, co, ts(t, P)],
                                     wvt_sb[:, co, :],
                                     start=(co == 0), stop=(co == 1))
            nc.vector.tensor_copy(
                vt_sb[:, 2 * u : 2 * u + 2, :],
                half[:].rearrange("p (t c) -> p t c", t=2))

        def s_pair(c, j):
            # S tiles 2j, 2j+1 -> one [P,2,512] PSUM slot, ONE exp over
            # [P,1024] into the pt store.
            s = c % 2
            ps = psS.tile([P, 2, MCH], F32, tag="ps", name="ps_s")
            for i in range(2):
                t = 2 * j + i
                nc.tensor.matmul(ps[:, i, :], k_sb[:, ts(t, P)],
                                 q_sb[:, ts(c, MCH)], start=True, stop=True)
            nc.scalar.activation(pt_sb[:, s, 2 * j : 2 * j + 2, :], ps[:], AF.Exp)

        def vp(c, t, out_ps, last=False):
            s = c % 2
            for co in range(2):
                nc.tensor.matmul(out_ps[:, co, :],
                                 vt_sb[:, t, ts(co, P)], pt_sb[:, s, t, :],
                                 start=(t == 0), stop=last)

        def den_tree(c):
            # dscr[0:8] = sum of the 4 tile-octs (pure reads of pt),
            # then fold 8 -> 4 -> 2 -> 1. All bf16 2x-mode wide adds.
            s = c % 2
            po = pt_sb[:, s, :, :].rearrange("p t m -> p (t m)")
            do = dscr[:].rearrange("p t m -> p (t m)")
            E = 8 * MCH
            nc.vector.tensor_add(do[:, 0:E], po[:, 0:E], po[:, E:2 * E])
            nc.vector.tensor_add(do[:, 0:E], do[:, 0:E], po[:, 2 * E:3 * E])
            nc.vector.tensor_add(do[:, 0:E], do[:, 0:E], po[:, 3 * E:4 * E])
            nc.vector.tensor_add(do[:, 0:E // 2], do[:, 0:E // 2],
                                 do[:, E // 2:E])
            nc.vector.tensor_add(do[:, 0:E // 4], do[:, 0:E // 4],
                                 do[:, E // 4:E // 2])
            nc.vector.tensor_add(do[:, 0:MCH], do[:, 0:MCH],
                                 do[:, MCH:2 * MCH])

        def den_mm(c):
            # ones^T @ acc: reduces over partitions AND broadcasts row-sums
            den_ps = psS.tile([P, 2, MCH], F32, tag="ps", name="ps_den")
            nc.tensor.matmul(den_ps[:, 0, :], ones_sb[:], dscr[:, 0, :],
                             start=True, stop=True)
            return den_ps

        def recip_of(den_ps):
            r = rcp.tile([P, MCH], F32, tag="rc")
            nc.vector.reciprocal_approx_fast(out=r[:], in_=den_ps[:, 0, :])
            return r

        def ep_mul(pout, r, co):
            o = osb.tile([P, MCH], F32, tag="osb")
            nc.vector.tensor_mul(o[:], pout[:, co, :], r[:])
            return o

        def ep_add_dma(c, o, co, q=nc.sync):
            nc.vector.tensor_add(o[:], o[:], ab2_sb[:, co, ts(c, MCH)])
            q.dma_start(out_v[:, co, ts(c, MCH)], o[:])

        def ab2():
            for co in range(2):
                nc.vector.tensor_scalar_add(ab2_sb[:, co, :], a_sb[:, co, :],
                                            bv_sb[:, co:co + 1])

        # ---------- prefix: projections + S/exp of chunk 0 ----------
        for e in range(8):
            psA = psS.tile([P, 2, MCH], F32, tag="ps", name="ps_proj_a")
            if e == 0:
                proj_q(0, psA[:, 1, :])
            elif e == 4:
                proj_q(1, psA[:, 1, :])
            proj_k(e, psA[:, 0, :])
            psB = psS.tile([P, 2, MCH], F32, tag="ps", name="ps_proj_b")
            proj_vt_pair(2 * e, psB[:, 0, :])
            proj_vt_pair(2 * e + 1, psB[:, 1, :])
            s_pair(0, 2 * e)
            s_pair(0, 2 * e + 1)
        ab2()

        # ---------- phases X_c: VP(c) + S/exp(c+1) + den(c) + ep(c) ----
        st = {}
        for c in range(NCH):
            # deferred S pairs of chunk c: PE work that covers the
            # ep_mul(c-1) drain of the psOut slot.
            if c >= 1:
                s_pair(c, 13)
                s_pair(c, 14)
                s_pair(c, 15)
            if 1 <= c < NCH - 1:
                psQ = psS.tile([P, 2, MCH], F32, tag="ps", name="ps_q")
                proj_q(c + 1, psQ[:, 0, :])
            den_tree(c)
            out_ps = psOut.tile([P, 2, MCH], F32, tag="out")
            for j in range(NT // 2):
                vp(c, 2 * j, out_ps)
                vp(c, 2 * j + 1, out_ps, last=(j == NT // 2 - 1))
                if c < NCH - 1 and j <= 12:
                    s_pair(c + 1, j)
                if j == 10:
                    st["den"] = den_mm(c)
                    st["rc"] = recip_of(st.pop("den"))
            # epilogue of chunk c, immediately after vp(c,31)
            rc = st.pop("rc")
            o0 = ep_mul(out_ps, rc, 0)
            o1 = ep_mul(out_ps, rc, 1)
            ep_add_dma(c, o0, 0, q=nc.sync)
            ep_add_dma(c, o1, 1, q=nc.scalar)


def _build():
    nc = bacc.Bacc("TRN2", target_bir_lowering=False, debug=False)
    a_d = nc.dram_tensor("a_s", [P, 2, MS], BF16, kind="ExternalInput").ap()
    p_d = nc.dram_tensor("p_s", [P, 2, HW], BF16, kind="ExternalInput").ap()
    wqt_d = nc.dram_tensor("wqt", [P, 2, CH], BF16, kind="ExternalInput").ap()
    wkt_d = nc.dram_tensor("wkt", [P, 2, CH], BF16, kind="ExternalInput").ap()
    wvt_d = nc.dram_tensor("wvt", [P, 2, C], BF16, kind="ExternalInput").ap()
    bq_d = nc.dram_tensor("bq", [CH, 1], F32, kind="ExternalInput").ap()
    bk_d = nc.dram_tensor("bk", [CH, 1], F32, kind="ExternalInput").ap()
    bv_d = nc.dram_tensor("bv", [P, 2], F32, kind="ExternalInput").ap()
    ones_d = nc.dram_tensor("onesm", [P, P], BF16, kind="ExternalInput").ap()
    out_d = nc.dram_tensor("out_s", [C, MS], F32, kind="ExternalOutput").ap()
    with tile.TileContext(nc) as tc:
        _emit(tc, out_d, a_d, p_d, wqt_d, wkt_d, wvt_d, bq_d, bk_d, bv_d, ones_d)
    nc.compile()
    return nc


def _get_prog():
    global _PROG
    if _PROG is None:
        _PROG = _build()
    return _PROG


def _ci_co(x):
    # [C, M] -> [ci, co, M] with C = co*128 + ci, cast bf16
    m = x.shape[1]
    return np.ascontiguousarray(
        x.reshape(2, P, m).transpose(1, 0, 2).astype(BT))


def kernel(**inputs):
    a = np.ascontiguousarray(np.asarray(inputs["a"], dtype=np.float32)).reshape(
        B, C, HW
    )
    p = np.ascontiguousarray(np.asarray(inputs["p"], dtype=np.float32)).reshape(
        B, C, HW
    )
    wqt = _ci_co(np.asarray(inputs["Wq"], dtype=np.float32).T)
    wkt = _ci_co(np.asarray(inputs["Wk"], dtype=np.float32).T)
    wvt = _ci_co(np.asarray(inputs["Wv"], dtype=np.float32).T)
    bq = np.ascontiguousarray(np.asarray(inputs["bq"], dtype=np.float32)).reshape(
        CH, 1
    )
    bk = np.ascontiguousarray(np.asarray(inputs["bk"], dtype=np.float32)).reshape(
        CH, 1
    )
    bv = np.ascontiguousarray(
        np.asarray(inputs["bv"], dtype=np.float32).reshape(2, P).T
    )

    nc = _get_prog()
    in_maps = []
    for core in range(NCORES):
        b, h = divmod(core, 2)
        in_maps.append(
            {
                "a_s": _ci_co(a[b, :, h * MS : (h + 1) * MS]),
                "p_s": _ci_co(p[b]),
                "wqt": wqt,
                "wkt": wkt,
                "wvt": wvt,
                "bq": bq,
                "bk": bk,
                "bv": bv,
                "onesm": _ONES,
            }
        )
    kwargs = {}
    if TRACE:
        kwargs["trace"] = True
        if TMPDIR:
            kwargs["tmpdir"] = TMPDIR
    res = run_bass_kernel_spmd(nc, in_maps, core_ids=list(range(NCORES)), **kwargs)
    global LAST_RESULT
    LAST_RESULT = res

    out = np.empty((B, C, HW), dtype=np.float32)
    for core in range(NCORES):
        b, h = divmod(core, 2)
        out[b, :, h * MS : (h + 1) * MS] = res.results[core]["out_s"]
    return out.reshape(B, C, H, W)
